# revision 113
# baseline (speedup 1.0000x reference)
"""GAT layer forward on 8 Trainium2 NeuronCores.

Math: out[b] = adj @ (input_h[b] @ W)   for b in 0..7
  input_h: [8, 4096, 256] f32, adj: [4096, 4096] f32 (0/1), W: [256, 64] f32

Strategy (contraction-dim sharding over the 8 cores):
  - Host pre-transposes adj -> adjT [m, i] (fp8, exact for 0/1) and reshapes
    input_h into per-m-tile blocks xt_r [t, b, f, 128].
  - Core c owns the m-slice [512c, 512c+512).  Phase A computes its h-shard
    h[b, m, o] = sum_f x[b, m, f] W[f, o] on the PE (fp32 matmuls, all 8
    batches accumulating column slices of one PSUM bank per m-tile), then
    splits h into scaled bf16 hi + scaled fp8 lo with two large DVE ops.
  - Phase B runs the SpMM partial
        outp[i, b*64+o] = sum_{m in shard} adjT[m, i] * h[b, m, o]
    as dense matmuls against a SINGLE adj copy holding adj * 2^-8 (exact
    fp8 subnormal, shipped pre-scaled): the bf16 hi pass streams 256*h
    (h_hi stored pre-scaled, exact in bf16) and the fp8 DoubleRow lo pass
    streams the 2^8-scaled residual — both recover adj*h exactly, and all
    six matmuls of an output chunk accumulate into one PSUM bank.
  - adj is loaded i-major (output-column pieces: 768 then 1024x3 then 256)
    so the last-arriving piece is only needed by the last output chunks:
    phase B starts as soon as h is ready (~14us) instead of when all of adj
    has landed (~18us); the first piece covers exactly the 6 prefilled
    chunks, keeping the critical xt stream as early as possible.
  - Dummy bf16 matmuls warm the PE clock (HAM) while the first loads fly;
    the first 6 chunks' pair-0 halves are emitted mid-phase-A to keep the
    PE fed while later xt/adj tiles are still in flight.
  - Host sums the 8 fp32 partials and reshapes to [b, i, o].

All dtype/perf-mode choices were validated on HW: fp8 lhsT x bf16 rhs is
legal and matches bf16 x bf16 exactly; ACT's fp8 cast preserves the 2^-8
subnormal; the hi/lo scheme gives ~4e-5 rel err end to end.
"""

import numpy as np
import concourse.mybir as mybir
import concourse.tile as tile
from concourse import bacc
from concourse import bass_utils

B, N, F_IN, F_OUT = 8, 4096, 256, 64
NCORES = 8
M_SHARD = N // NCORES  # 512
P = 128
MT = M_SHARD // P  # 4 m-tiles per core
FT = F_IN // P     # 2 f-tiles
NBO = B * F_OUT    # 512 = packed (b, o) free dim
IC = N // P        # 32 output row chunks

_nc_cache = None


def _emit(nc, tc, adjt, xt, w, outp):
    f32 = mybir.dt.float32
    bf16 = mybir.dt.bfloat16
    fp8 = mybir.dt.float8e4

    with (
        tc.tile_pool(name="const", bufs=1) as cpool,
        tc.tile_pool(name="work", bufs=6) as wpool,
        tc.tile_pool(name="psum_o", bufs=6, space="PSUM") as ppool,
        tc.tile_pool(name="psum_h", bufs=2, space="PSUM") as phpool,
    ):
        # W: [256, 64] -> sbuf [128, 2, 64]
        w_sb = cpool.tile([P, FT, F_OUT], f32, tag="w")
        nc.sync.dma_start(out=w_sb[:], in_=w.rearrange("(t p) o -> p t o", p=P))

        # Input DMA order is chosen so the PE never waits longer than it has
        # to: xt per m-tile (phase A consumes t-major), with the adj pair
        # tiles interleaved so phase B's early matmuls can start while the
        # tail of xt/adj is still in flight.
        xt_sb = []  # per t: [128(f), B, FT, 128(m)]
        adj_sb = [None, None]
        adjlo_sb = [None, None]

        def load_xt(t, parts=2):
            xb = cpool.tile([P, B, FT, P], f32, tag=f"xtt{t}", name=f"xtt{t}")
            bw = B // parts
            for hb in range(parts):
                bs = slice(hb * bw, (hb + 1) * bw)
                nc.sync.dma_start(
                    out=xb[:, bs],
                    in_=xt[t, bs].rearrange("b (g p) m -> p b g m", p=P),
                )
            xt_sb.append(xb)

        for pr in range(2):
            adjlo_sb[pr] = cpool.tile(
                [P, 2, N], fp8, tag=f"adjlo{pr}", name=f"adjlo{pr}"
            )

        def load_adj_piece(c0, c1):
            # The ONLY adj copy on chip is the host-scaled adj*2^-8 (exact
            # fp8 subnormal): the hi pass multiplies it by 256*h (h_hi is
            # stored pre-scaled, exact in bf16) and the DoubleRow lo pass by
            # the 2^8-scaled residual — both recover adj*h exactly.  Loaded
            # i-major (output-column pieces) so the last-arriving piece is
            # only needed by the last output chunks.
            cs = slice(c0, c1)
            for pr in range(2):
                for j in range(2):
                    t = 2 * pr + j
                    nc.sync.dma_start(
                        out=adjlo_sb[pr][:, j, cs], in_=adjt[t * P : (t + 1) * P, cs]
                    )

        # PE warm-up: dummy bf16 matmuls on a zeroed tile while the first
        # xt DMA is in flight, so phase A's fp32 matmuls run at full clock
        dummy = cpool.tile([P, NBO], bf16, tag="dummy")
        nc.gpsimd.memset(dummy[:], 0.0)
        dps = ppool.tile([P, NBO], f32, tag="po", name="dps")
        for wi in range(8):
            nc.tensor.matmul(
                dps[:], dummy[:, :P], dummy[:], start=(wi == 0), stop=(wi == 7)
            )

        load_xt(0, parts=4)
        load_xt(1)
        load_adj_piece(0, 768)
        load_xt(2)
        load_xt(3)
        load_adj_piece(768, 1792)
        load_adj_piece(1792, 2816)
        load_adj_piece(2816, 3840)
        load_adj_piece(3840, 4096)

        def chunk_half(po, ic, pr, start):
            for j in range(2):
                t = 2 * pr + j
                lhs = adjlo_sb[pr][:, j, ic * P : (ic + 1) * P]
                nc.tensor.matmul(
                    po[:], lhs, h_hi[t][:], start=start, stop=False
                )
                start = False
            nc.tensor.matmul(
                po[:],
                adjlo_sb[pr][:, :, ic * P : (ic + 1) * P],
                h_lo8[pr][:],
                start=False,
                stop=(pr == MT // 2 - 1),
                perf_mode=mybir.MatmulPerfMode.DoubleRow,
            )

        # Phase A: h = x @ W per (m-tile, b); split into bf16 hi and
        # 2^8-scaled fp8 lo (consumed by the DoubleRow lo pass in phase B).
        h_hi = [
            cpool.tile([P, NBO], bf16, tag=f"hhi{t}", name=f"hhi{t}")
            for t in range(MT)
        ]
        h_lo8 = [
            cpool.tile([P, 2, NBO], fp8, tag=f"hlo8p{pr}", name=f"hlo8p{pr}")
            for pr in range(MT // 2)
        ]
        PRE = 6  # chunks whose pair-0 half is emitted mid-phase-A
        pre_po = {}

        def phase_a(trange):
            # all 8 batches of a t-tile accumulate into one [128, 512] PSUM
            # bank (disjoint column slices), so the hi/lo split is 3 large
            # DVE ops per tile instead of 24 small chained ones
            for t in trange:
                ph = phpool.tile([P, NBO], f32, tag="ph")
                for b in range(B):
                    for f in range(FT):
                        nc.tensor.matmul(
                            ph[:, b * F_OUT : (b + 1) * F_OUT],
                            xt_sb[t][:, b, f],
                            w_sb[:, f],
                            start=(f == 0),
                            stop=(f == FT - 1),
                        )
                nc.vector.tensor_scalar_mul(h_hi[t][:], ph[:], 256.0)
                nc.vector.scalar_tensor_tensor(
                    out=h_lo8[t // 2][:, t % 2, :],
                    in0=ph[:],
                    scalar=256.0,
                    in1=h_hi[t][:],
                    op0=mybir.AluOpType.mult,
                    op1=mybir.AluOpType.subtract,
                )

        phase_a(range(2))
        # pair-0 halves of the first PRE chunks fill the PE while phase A's
        # t=2/3 still wait on their xt DMAs (head-of-line blocking fix)
        for ic in range(PRE):
            po = ppool.tile([P, NBO], f32, tag="po", name=f"pre_po{ic}")
            pre_po[ic] = po
            chunk_half(po, ic, 0, start=True)
        phase_a(range(2, MT))

        # Phase B: per output chunk, six matmuls accumulate into one PSUM
        # bank — hi passes (bf16) and scaled lo passes (fp8 DoubleRow).
        ot = None
        for ic in range(IC):
            if ic in pre_po:
                po = pre_po[ic]
                chunk_half(po, ic, 1, start=False)
            else:
                po = ppool.tile([P, NBO], f32, tag="po")
                chunk_half(po, ic, 0, start=True)
                chunk_half(po, ic, 1, start=False)
            # stage to SBUF on DVE (ACT is reserved for adjlo derivation);
            # store in 512KB pairs, final pair as singles to cut the tail
            j = ic % 2
            if j == 0:
                ot = wpool.tile([P, 2, NBO], f32, tag="ot")
            if ic % 2 == 0:
                nc.vector.tensor_copy(out=ot[:, j], in_=po[:])
            else:
                nc.scalar.copy(out=ot[:, j], in_=po[:])
            if ic >= IC - 2:
                nc.sync.dma_start(
                    out=outp[ic * P : (ic + 1) * P, :], in_=ot[:, j]
                )
            elif j == 1:
                nc.sync.dma_start(
                    out=outp[(ic - 1) * P : (ic + 1) * P, :].rearrange(
                        "(c p) n -> p c n", p=P
                    ),
                    in_=ot[:],
                )


def _build():
    global _nc_cache
    if _nc_cache is not None:
        return _nc_cache
    nc = bacc.Bacc("TRN2", target_bir_lowering=False, debug=False, num_devices=NCORES)
    adjt = nc.dram_tensor(
        "adjt", [M_SHARD, N], mybir.dt.float8e4, kind="ExternalInput"
    ).ap()
    xt = nc.dram_tensor(
        "xt", [MT, B, F_IN, P], mybir.dt.float32, kind="ExternalInput"
    ).ap()
    w = nc.dram_tensor("w", [F_IN, F_OUT], mybir.dt.float32, kind="ExternalInput").ap()
    outp = nc.dram_tensor(
        "outp", [N, NBO], mybir.dt.float32, kind="ExternalOutput"
    ).ap()
    with tile.TileContext(nc) as tc:
        _emit(nc, tc, adjt, xt, w, outp)
    nc.compile()
    _nc_cache = nc
    return nc


def _in_maps(input_h, adj, W):
    fp8np = mybir.dt.np(mybir.dt.float8e4)
    # ship adj pre-scaled by 2^-8 — exact (fp8 subnormal), and the only
    # adj copy the kernel needs
    adjt8 = (np.ascontiguousarray(adj.T) * (1.0 / 256.0)).astype(fp8np)
    # xt_r[c][t, b, f, p] = input_h[b, c*512 + t*128 + p, f]
    xt_r = input_h.reshape(B, NCORES, MT, P, F_IN).transpose(1, 2, 0, 4, 3)
    w = np.ascontiguousarray(W.astype(np.float32))
    maps = []
    for c in range(NCORES):
        sl = slice(c * M_SHARD, (c + 1) * M_SHARD)
        maps.append(
            {
                "adjt": np.ascontiguousarray(adjt8[sl, :]),
                "xt": np.ascontiguousarray(xt_r[c]),
                "w": w,
            }
        )
    return maps


def kernel(input_h, adj, W, _profile=False):
    nc = _build()
    maps = _in_maps(
        np.asarray(input_h, dtype=np.float32),
        np.asarray(adj, dtype=np.float32),
        np.asarray(W, dtype=np.float32),
    )
    # The device occasionally throws a transient NRT_EXEC_UNIT_UNRECOVERABLE;
    # a straight retry recovers it.
    last_err = None
    for _ in range(3):
        try:
            res = bass_utils.run_bass_kernel_spmd(
                nc, maps, core_ids=list(range(NCORES)), trace=_profile
            )
            break
        except Exception as e:
            last_err = e
    else:
        raise last_err
    total = res.results[0]["outp"].copy()
    for c in range(1, NCORES):
        total += res.results[c]["outp"]
    out = np.ascontiguousarray(total.reshape(N, B, F_OUT).transpose(1, 0, 2))
    if _profile:
        return out, res
    return out
